# revision 39
# baseline (speedup 1.0000x reference)
"""Trainium2 Bass kernel for nn_MHA_42391327211690 (v2, bf16 dataflow).

MHA: B=1, S=2048, E=2048, H=32 q-heads, HKV=8 kv-heads, D=64, RoPE(rot=64,
GPT-NeoX style) on q/k, causal GQA attention, out-projection with bias.

Distribution (8 NeuronCores, tensor-parallel by heads):
  - core i computes q-heads 4i..4i+3 and kv-head i (Wqkv column-sharded),
  - attention entirely local (GQA groups align with the shard),
  - AllToAll (bf16) redistributes ctx^T from head-sharded to seq-sharded,
  - out-projection computed per-core for its 256-row sequence slice
    (weights replicated), host concatenates the slices.

All matmuls run in bf16 (full PE rate at any free size, half the DMA and
SBUF of fp32); PSUM accumulation stays fp32, so the dominant error is the
bf16 quantization of the operands (~0.4%), far inside the 2e-2 gate.

Schedule notes (v7, 265us vs the 365us fp32r baseline; measured rel err
4.0e-3 on HW, identical to the CoreSim prediction):
  - weight/x loads are merged multi-dim DMAs (the sync engine costs ~650ns
    per dma_start, so many small loads pace the kernel),
  - scores/ctx matmuls causally trimmed at 128-column granularity; exp
    reads only PSUM regions written this iteration,
  - softmax denominator: ones-column appended to V; its reciprocal is
    broadcast across partitions via a DRAM-bounce stride-0 DMA,
  - persistent SBUF tensors written in phases (q, roped k) are split into
    per-s-group tiles: dependency tracking is whole-tile, so a single tile
    would stall attention on the last s-group's write,
  - A2A payloads are bf16 but the DRAM tiles are declared f32 at half
    width (ENCD sizes collectives as 4-byte elements),
  - the A2A-dependent ctxF loads issue from the gpsimd queue, which
    already blocks on the collective wait - on any other queue the list
    scheduler hoists them into that engine's critical stream,
  - group normalize/staging is software-pipelined one group late so the
    next group's masks never queue behind the reciprocal chain,
  - out-proj pass0 (head-pair 0's contraction half) runs during A2A#1.

HW-vs-sim pitfalls learned (see memory): custom-DVE ops (reciprocal_*)
return NaN on HW when their input AP has a partition offset - stage to
partition 0 first; gpsimd partition_broadcast is broken on HW.
"""

from contextlib import ExitStack

import numpy as np
import ml_dtypes

import concourse.bass as bass
import concourse.bacc as bacc
import concourse.tile as tile
from concourse import mybir
from concourse.bass_utils import run_bass_kernel_spmd

F32 = mybir.dt.float32
BF16 = mybir.dt.bfloat16
AF = mybir.ActivationFunctionType
ALU = mybir.AluOpType

B, E = 1, 2048
H, HKV, D = 32, 8, 64
ROT, BASE = 64, 10000.0
NCORES = 8
HL = H // NCORES            # 4 local q heads
OPL = (HL + 2) * D          # 384 local qkv output rows (q | k | v)
SCALE = float(D) ** -0.5


def build_nc(S=2048, n_cores=NCORES):
    """Build the SPMD Bass program (identical on every core)."""
    SEG = S // n_cores      # per-core output sequence slice
    NT = S // 128           # t-blocks (key blocks)
    NG = S // 512           # sq groups of 512
    NE = E // 128           # contraction tiles for qkv / out proj
    NSG = S // 512

    nc = bacc.Bacc("TRN2", target_bir_lowering=False, debug=False,
                   num_devices=n_cores)

    xT = nc.dram_tensor("xT", [E, S], BF16, kind="ExternalInput")
    wqkvT = nc.dram_tensor("wqkvT", [E, OPL], BF16, kind="ExternalInput")
    bqkv = nc.dram_tensor("bqkv", [OPL, 1], F32, kind="ExternalInput")
    cc_d = nc.dram_tensor("cc", [128, S], BF16, kind="ExternalInput")
    ss_d = nc.dram_tensor("ss", [128, S], BF16, kind="ExternalInput")
    triu_d = nc.dram_tensor("triu", [128, 128], BF16, kind="ExternalInput")
    id_d = nc.dram_tensor("ident", [128, 64], BF16, kind="ExternalInput")
    woT = nc.dram_tensor("woT", [E, E], BF16, kind="ExternalInput")
    outb_d = nc.dram_tensor("outb", [128, E], F32, kind="ExternalInput")
    outS = nc.dram_tensor("outS", [SEG, E], F32, kind="ExternalOutput")

    with tile.TileContext(nc) as tc, ExitStack() as ctx:
        consts = ctx.enter_context(tc.tile_pool(name="consts", bufs=1))
        triu = consts.tile([128, 128], BF16)
        bq = [consts.tile([128, 1], F32, tag=f"bq{j}", name=f"bq{j}") for j in range(3)]

        # persistent qkv activations (bf16)
        qkv_pool = ctx.enter_context(tc.tile_pool(name="qkv", bufs=1))
        # per-sg q tiles: a single [128,S] tile would give every score matmul
        # a whole-tile dependency on the last s-group's rope write
        q_sb = [[qkv_pool.tile([128, 512], BF16, tag=f"q{i}_{sg}", name=f"qsb{i}_{sg}")
                 for sg in range(S // 512)] for i in range(HL // 2)]
        kv_sb = qkv_pool.tile([128, S], BF16, tag="kv")    # k rows 0:64, v rows 64:128
        # roped+duplicated k, one tile per s-group: attention's dependency on
        # k is then per-group (a single [128,S] tile would make every score
        # matmul wait for the final s-group's dup DMA)
        kd = [qkv_pool.tile([128, 512], BF16, tag=f"kd{sg}", name=f"kd{sg}")
              for sg in range(S // 512)]
        v_pool = ctx.enter_context(tc.tile_pool(name="vsb", bufs=1))
        v_sb = [v_pool.tile([128, D + 1], BF16, tag=f"v{t}", name=f"vsb{t}") for t in range(NT)]

        # ---------------- Phase A: QKV + RoPE + V transpose -------------
        with tc.tile_pool(name="xw", bufs=1) as xw_pool, \
             tc.tile_pool(name="ropet", bufs=2) as rope_pool, \
             tc.tile_pool(name="psqkv", bufs=2, space="PSUM") as ps_qkv, \
             tc.tile_pool(name="psvt", bufs=2, space="PSUM") as ps_vt:

            cc = xw_pool.tile([128, S], BF16)
            ss = xw_pool.tile([128, S], BF16)
            ident = xw_pool.tile([128, 64], BF16)
            # all 16 e-tiles of W / of one s-group of x live side by side in
            # one tile, loaded by a single multi-dim DMA: the sync engine
            # spends ~650ns per dma_start, so 48 small loads would make it a
            # phase-A co-pacer
            wq_sb = xw_pool.tile([128, NE * OPL], BF16, name="wqsb")
            xs = {}

            def _x_group(sg, nchunk):
                xs[sg] = xw_pool.tile([128, NE * 512], BF16, tag="x",
                                      bufs=2, name=f"xs{sg}")
                epc = NE // nchunk
                for k in range(nchunk):
                    dst = xs[sg][:, k * epc * 512:(k + 1) * epc * 512]
                    nc.sync.dma_start(
                        bass.AP(dst.tensor, dst.offset,
                                [dst.ap[0], [512, epc], [1, 512]]),
                        bass.AP(xT[:].tensor, (k * epc * 128) * S + sg * 512,
                                [[S, 128], [128 * S, epc], [1, 512]]))

            def _wq(k, epc):
                dst = wq_sb[:, k * epc * OPL:(k + 1) * epc * OPL]
                nc.sync.dma_start(
                    bass.AP(dst.tensor, dst.offset,
                            [dst.ap[0], [OPL, epc], [1, OPL]]),
                    bass.AP(wqkvT[:].tensor, k * epc * 128 * OPL,
                            [[OPL, 128], [128 * OPL, epc], [1, OPL]]))

            # interleave weight/x chunks so matmul e=0 can start early
            _wq(0, 4)
            _x_group(0, 4)
            for j in range(3):
                nc.sync.dma_start(bq[j][:], bqkv[j * 128:(j + 1) * 128, :])
            for k in range(1, 4):
                _wq(k, 4)
            nc.sync.dma_start(cc[:], cc_d[:])
            nc.sync.dma_start(ss[:], ss_d[:])
            nc.sync.dma_start(triu[:], triu_d[:])
            nc.sync.dma_start(ident[:], id_d[:])

            # warm the ACT exp table so the first attention exp call does
            # not pay the ~1.3us ACT_TABLE_LOAD mid-pipeline
            warm = xw_pool.tile([128, 1], F32, tag="warm")
            nc.scalar.activation(warm[:], bq[0][:], AF.Exp, scale=0.0)

            for sg in range(NSG):
                if sg + 1 < NSG:
                    _x_group(sg + 1, 2)
                sgs = slice(sg * 512, (sg + 1) * 512)
                ps = ps_qkv.tile([128, 1536], F32, tag="qkvps")
                for e in range(NE):
                    for j in range(3):
                        nc.tensor.matmul(
                            ps[:, j * 512:(j + 1) * 512],
                            wq_sb[:, e * OPL + j * 128:e * OPL + (j + 1) * 128],
                            xs[sg][:, e * 512:(e + 1) * 512],
                            start=(e == 0), stop=(e == NE - 1))
                # kv first: the V transposes (PE) and k-rope depend on it
                for j in (2, 0, 1):
                    dst = kv_sb[:, sgs] if j == 2 else q_sb[j][sg][:]
                    nc.scalar.activation(
                        dst, ps[:, j * 512:(j + 1) * 512],
                        AF.Identity, bias=bq[j][:], scale=1.0)
                # ---- V transpose first (kv-eviction dependent only), so the
                # PE's transposes never queue behind this sg's rope on DVE
                for c in range(4):
                    t = sg * 4 + c
                    pvt = ps_vt.tile([128, 64], BF16, tag="vt")
                    nc.tensor.transpose(
                        pvt[:],
                        kv_sb[64:128, sg * 512 + c * 128: sg * 512 + (c + 1) * 128],
                        ident[64:128, :])
                    nc.vector.memset(v_sb[t][:, 64:65], 1.0)
                    nc.vector.tensor_copy(v_sb[t][:, 0:64], pvt[:])
                # ---- RoPE: swapped halves built via SBUF->SBUF DMA on the
                # scalar HWDGE queue (DVE requires equal partitions)
                for qi in range(HL // 2):
                    qt = q_sb[qi][sg]
                    qs = rope_pool.tile([128, 512], BF16, tag="qs")
                    for b in range(4):
                        nc.sync.dma_start(
                            qs[b * 32:(b + 1) * 32, :],
                            qt[(b ^ 1) * 32:((b ^ 1) + 1) * 32, :])
                    t1 = rope_pool.tile([128, 512], BF16, tag="t1")
                    t2 = rope_pool.tile([128, 512], BF16, tag="t2")
                    nc.vector.tensor_mul(t1[:], qt[:], cc[:, sgs])
                    nc.vector.tensor_mul(t2[:], qs[:], ss[:, sgs])
                    nc.vector.tensor_add(qt[:], t1[:], t2[:])
                # k: rows 0:64 of kv_sb -> roped into kd[sg], then dup'd
                ks = rope_pool.tile([64, 512], BF16, tag="ks")
                for b in range(2):
                    nc.sync.dma_start(
                        ks[b * 32:(b + 1) * 32, :],
                        kv_sb[(b ^ 1) * 32:((b ^ 1) + 1) * 32, sgs])
                t1 = rope_pool.tile([64, 512], BF16, tag="kt1")
                t2 = rope_pool.tile([64, 512], BF16, tag="kt2")
                nc.vector.tensor_mul(t1[:], kv_sb[0:64, sgs], cc[0:64, sgs])
                nc.vector.tensor_mul(t2[:], ks[:], ss[0:64, sgs])
                nc.vector.tensor_add(kd[sg][0:64, :], t1[:], t2[:])
                nc.sync.dma_start(kd[sg][64:128, :], kd[sg][0:64, :])

        # ---------------- Phase B+C: attention + out projection ---------
        ctx_pool = ctx.enter_context(tc.tile_pool(name="ctxsb", bufs=1))
        ctx_sb = [ctx_pool.tile([64, S], BF16, tag=f"c{i}", name=f"ctxsb{i}")
                  for i in range(HL)]
        wo_pool = ctx.enter_context(tc.tile_pool(name="wo", bufs=1, side="right"))
        wo_sb = [wo_pool.tile([128, E], BF16, tag=f"wo{f}", name=f"wosb{f}")
                 for f in range(NE)]
        cf_pool = ctx.enter_context(tc.tile_pool(name="cf", bufs=1))
        outb = cf_pool.tile([128, E], F32)
        ctxF = [cf_pool.tile([128, SEG], BF16, tag=f"cf{f}", name=f"cfsb{f}")
                for f in range(NE)]
        out_pool = ctx.enter_context(tc.tile_pool(name="osb", bufs=1))
        out_sb = [out_pool.tile([128, E], F32, tag=f"ot{s}", name=f"osb{s}")
                  for s in range(SEG // 128)]

        # A2A payload is bf16 but the DRAM tiles are declared f32 with
        # half the columns: ENCD sizes collective transfers as 4-byte
        # elements, so a bf16-typed collective moves 2x the buffer and
        # corrupts it. Same bytes either way; SBUF-side DMAs bitcast.
        dram = ctx.enter_context(tc.tile_pool(name="dram", bufs=1, space="DRAM"))
        a2a_in = [dram.tile([n_cores * 128, SEG // 2], F32, tag=f"ai{p}", name=f"a2ain{p}")
                  for p in range(2)]
        a2a_out = [dram.tile([n_cores * 128, SEG // 2], F32, tag=f"ao{p}", name=f"a2aout{p}")
                   for p in range(2)]

        # out-proj weights + bias stream during attention (sync queue)
        for f in range(NE):
            nc.sync.dma_start(wo_sb[f][:], woT[f * 128:(f + 1) * 128, :])
        nc.sync.dma_start(outb[:], outb_d[:])

        with tc.tile_pool(name="exps", bufs=2) as exps_pool, \
             tc.tile_pool(name="rcp", bufs=2) as rcp_pool, \
             tc.tile_pool(name="pss", bufs=2, space="PSUM") as ps_s, \
             tc.tile_pool(name="psctx", bufs=2, space="PSUM") as ps_ctx:

            def _epi2(hp, g, cu, rb):
                """Group epilogue part 2: normalize + A2A staging. Emitted
                one group late so nothing queues behind the rcp chain; reads
                the promptly-evicted cu so the pc PSUM banks free early."""
                gsl = slice(g * 512, (g + 1) * 512)
                nc.vector.tensor_mul(ctx_sb[2 * hp][:, gsl],
                                     cu[:, 0:512], rb[:, 0:512])
                nc.vector.tensor_mul(ctx_sb[2 * hp + 1][:, gsl],
                                     cu[:, 512:1024], rb[:, 512:1024])
                # stage this group's A2A slices (dest cores 2g, 2g+1)
                for jj in (2 * g, 2 * g + 1):
                    for h2 in range(2):
                        nc.sync.dma_start(
                            a2a_in[hp][jj * 128 + h2 * 64: jj * 128 + (h2 + 1) * 64, :]
                            .bitcast(BF16),
                            ctx_sb[2 * hp + h2][:, jj * SEG:(jj + 1) * SEG])

            pend = None
            for hp in range(HL // 2):
                for g in range(NG):
                    qt = q_sb[hp][g]
                    pc_e = ps_ctx.tile([D + 1, 512], F32, tag="ctx_e")
                    pc_o = ps_ctx.tile([D + 1, 512], F32, tag="ctx_o")
                    ntb = 4 * g + 4
                    for t in range(ntb):
                        kdt = kd[t // 4]
                        ts_ = slice((t % 4) * 128, (t % 4 + 1) * 128)
                        j = t - 4 * g
                        off = 0 if j < 0 else j * 128   # first valid query col
                        pss = ps_s.tile([128, 1024], F32, tag="s")
                        nc.tensor.matmul(
                            pss[:, off:512], kdt[0:64, ts_],
                            qt[0:64, off:512], start=True, stop=True)
                        nc.tensor.matmul(
                            pss[:, 512 + off:1024], kdt[64:128, ts_],
                            qt[64:128, off:512], start=True, stop=True)
                        ex = exps_pool.tile([128, 1024], BF16, tag="e")
                        if off == 0:
                            nc.scalar.activation(ex[:], pss[:], AF.Exp, scale=SCALE)
                        else:
                            # trimmed: only read PSUM regions written above
                            nc.scalar.activation(ex[:, off:512], pss[:, off:512],
                                                 AF.Exp, scale=SCALE)
                            nc.scalar.activation(ex[:, 512 + off:1024],
                                                 pss[:, 512 + off:1024],
                                                 AF.Exp, scale=SCALE)
                        if j >= 0:
                            for h2 in range(2):
                                sl = slice(h2 * 512 + j * 128, h2 * 512 + (j + 1) * 128)
                                nc.vector.tensor_mul(ex[:, sl], ex[:, sl], triu[:])
                        nc.tensor.matmul(pc_e[:, off:512], v_sb[t][:],
                                         ex[:, off:512],
                                         start=(t == 0), stop=(t == ntb - 1))
                        nc.tensor.matmul(pc_o[:, off:512], v_sb[t][:],
                                         ex[:, 512 + off:1024],
                                         start=(t == 0), stop=(t == ntb - 1))
                    # epilogue part 1: denominator row straight out of PSUM
                    # to partition 0 (custom-DVE ops misbehave on HW with
                    # partition-offset inputs), reciprocal, then partition-
                    # broadcast via doubling SBUF->SBUF DMAs (sync queue);
                    # bulk ctx eviction runs after, off the rcp critical path
                    den = rcp_pool.tile([1, 1024], F32, tag="den")
                    nc.vector.tensor_copy(den[0:1, 0:512], pc_e[64:65, :])
                    nc.vector.tensor_copy(den[0:1, 512:1024], pc_o[64:65, :])
                    rb = rcp_pool.tile([64, 1024], F32, tag="rb")
                    nc.vector.reciprocal_approx_fast(rb[0:1, :], den[:])
                    # evict ctx to SBUF promptly: muls run one group late,
                    # and holding pc (bufs=2) until then stalls group g+2's
                    # accumulation start by the whole broadcast latency
                    cu = rcp_pool.tile([D, 1024], F32, tag="cu")
                    nc.vector.tensor_copy(cu[:, 0:512], pc_e[0:64, :])
                    nc.vector.tensor_copy(cu[:, 512:1024], pc_o[0:64, :])
                    # partition-broadcast rb row 0 via a DRAM bounce (2 DMAs;
                    # ~13us round trip, but pipelined one group late so only
                    # group 0's latency is exposed -- measured equal-or-better
                    # than the 6-hop SBUF doubling chain across runs)
                    rbd = dram.tile([1, 1024], F32, tag="rbd", bufs=2)
                    nc.sync.dma_start(rbd[:], rb[0:1, :])
                    rsrc = rbd[:]
                    nc.sync.dma_start(
                        rb[0:64, :],
                        bass.AP(rsrc.tensor, rsrc.offset, [[0, 64]] + rsrc.ap[1:]))
                    if pend is not None:
                        _epi2(*pend)
                    pend = (hp, g, cu, rb)
                _epi2(*pend)
                pend = None
                with tc.high_priority():
                    nc.gpsimd.collective_compute(
                        "AllToAll", ALU.bypass,
                        replica_groups=[list(range(n_cores))],
                        ins=[a2a_in[hp][:]], outs=[a2a_out[hp][:]])

        # ---------------- out projection --------------------------
        with tc.tile_pool(name="pso", bufs=2, space="PSUM") as ps_o:
            def _ctxF(par):
                # gpsimd queue: it already blocks on the collective wait, so
                # these A2A-dependent loads cannot be hoisted by the scheduler
                # into another engine's critical stream (scalar/sync heads
                # would stall ~25us waiting for the collective)
                for f in range(par, NE, 2):
                    c = f // 2
                    nc.gpsimd.dma_start(
                        ctxF[f][:],
                        a2a_out[par][c * 128:(c + 1) * 128, :].bitcast(BF16))

            _ctxF(0)
            _ctxF(1)
            # pass 0: head-pair-0 partials (needs only A2A#0), + bias; runs
            # while A2A#1 is in flight. pass 1 accumulates on top.
            for p in range(2):
                for st in range(SEG // 128):
                    for eg in range(4):
                        po = ps_o.tile([128, 512], F32, tag="o")
                        fs = [f for f in range(NE) if f % 2 == p]
                        for fi, f in enumerate(fs):
                            nc.tensor.matmul(
                                po[:],
                                ctxF[f][:, st * 128:(st + 1) * 128],
                                wo_sb[f][:, eg * 512:(eg + 1) * 512],
                                start=(fi == 0), stop=(fi == len(fs) - 1))
                        osl = out_sb[st][:, eg * 512:(eg + 1) * 512]
                        if p == 0:
                            nc.vector.scalar_tensor_tensor(
                                osl, po[:], 1.0,
                                outb[:, eg * 512:(eg + 1) * 512], ALU.mult, ALU.add)
                        else:
                            nc.vector.scalar_tensor_tensor(
                                osl, po[:], 1.0, osl, ALU.mult, ALU.add)
                    if p == 1:
                        nc.sync.dma_start(outS[st * 128:(st + 1) * 128, :],
                                          out_sb[st][:])

    nc.compile()
    return nc


def shard_inputs(hidden_states, Wqkv_w, Wqkv_b, out_w, out_b, S=2048,
                 n_cores=NCORES):
    """Host-side sharding: returns per-core input maps (bf16 operands)."""
    bf16 = ml_dtypes.bfloat16
    x = np.asarray(hidden_states, np.float32).reshape(S, E)
    xT = np.ascontiguousarray(x.T).astype(bf16)
    Wqkv_w = np.asarray(Wqkv_w, np.float32)
    Wqkv_b = np.asarray(Wqkv_b, np.float32)
    woT = np.ascontiguousarray(np.asarray(out_w, np.float32).T).astype(bf16)
    outb = np.ascontiguousarray(np.broadcast_to(
        np.asarray(out_b, np.float32).reshape(1, E), (128, E)))

    inv = (1.0 / (BASE ** (np.arange(0, ROT, 2, dtype=np.float64) / ROT)))
    t = np.arange(S, dtype=np.float64)
    freqs = np.outer(t, inv)                      # [S, 32]
    cT = np.cos(freqs).T.astype(np.float32)       # [32, S]
    sT = np.sin(freqs).T.astype(np.float32)
    cc = np.tile(cT, (4, 1)).astype(bf16)         # [128, S]
    ss = np.concatenate([-sT, sT, -sT, sT], axis=0).astype(bf16)
    triu = (np.arange(128)[:, None] <= np.arange(128)[None, :]).astype(bf16)
    ident = np.vstack([np.eye(64, dtype=np.float32)] * 2).astype(bf16)

    in_maps = []
    for i in range(n_cores):
        hq = H // n_cores
        wq = Wqkv_w[i * hq * D:(i + 1) * hq * D]          # [256, E]
        wk = Wqkv_w[H * D + i * D: H * D + (i + 1) * D]   # [64, E]
        wv = Wqkv_w[(H + HKV) * D + i * D: (H + HKV) * D + (i + 1) * D]
        w_local = np.concatenate([wq, wk, wv], axis=0)    # [384, E]
        b_local = np.concatenate([
            Wqkv_b[i * hq * D:(i + 1) * hq * D],
            Wqkv_b[H * D + i * D: H * D + (i + 1) * D],
            Wqkv_b[(H + HKV) * D + i * D: (H + HKV) * D + (i + 1) * D]])
        in_maps.append({
            "xT": xT,
            "wqkvT": np.ascontiguousarray(w_local.T).astype(bf16),
            "bqkv": np.ascontiguousarray(b_local.reshape(OPL, 1)),
            "cc": cc, "ss": ss, "triu": triu, "ident": ident,
            "woT": woT, "outb": outb,
        })
    return in_maps


def assemble(results, S=2048, n_cores=NCORES):
    out = np.concatenate([r["outS"] for r in results], axis=0)
    return out.reshape(B, S, E).astype(np.float32)


_NC_CACHE = {}


def _get_nc(S=2048):
    if S not in _NC_CACHE:
        _NC_CACHE[S] = build_nc(S=S)
    return _NC_CACHE[S]


def kernel(hidden_states, Wqkv_w, Wqkv_b, out_w, out_b, _trace=False):
    in_maps = shard_inputs(hidden_states, Wqkv_w, Wqkv_b, out_w, out_b)
    nc = _get_nc()
    res = run_bass_kernel_spmd(nc, in_maps, core_ids=list(range(NCORES)),
                               trace=_trace)
    out = assemble(res.results)
    if _trace:
        kernel.last_results = res
    return out
